# revision 54
# baseline (speedup 1.0000x reference)
"""Trainium2 Bass kernel for nn_AttnBlock_16887811407979 (sparse attention).

Strategy: 8-way sequence-parallel SPMD (each core handles a 256-query
slice, all heads), no collectives. The sparse gather is densified: the
host converts (attendable_indices, valid_indices_mask) into a dense
count matrix C[n, q], so softmax-over-slots == count-weighted dense
softmax:
    W[n,q] = C[n,q] * exp(S^T[n,q]);  O[q] = (W^T V) / sum_n W[n,q].

Orchestration (vs the original 147us version):
  - x shipped bf16 (halves the critical head DMA); kt/qt/vt/wt bf16
  - single ACT table set (natural_log_exp): GN 1/sqrt(var+eps) via
    exp(-0.5 ln(var+eps)), softmax 1/s via exp(-ln(s 2^-32) - 32 ln 2)
    (the Ln LUT is only accurate for |ln x| < ~40) -- kills the sqrt
    table load and the 4.4us single-partition DVE reciprocals
  - K bias dropped (softmax-invariant)
  - phase 1 fuses the K/V conv matmul stream with BOTH passes'
    score->exp->cnt-mul chains, so the PE never idles long enough for
    HAM to re-throttle; the 32 masked-weight tiles persist in SBUF
    (sharing buffers with the consumed x tiles)
  - phase 2 is a pure OV matmul streak; O^T accumulators are copied
    out of PSUM immediately (bf16) and softmax normalization runs on
    SBUF overlapped with the following matmuls; proj head-blocks
    accumulate while the pass-1 normalization finishes
"""
import sys
import types
import contextlib

sys.path.insert(0, '/opt/trn_rl_repo')
sys.path.insert(0, '/root/.axon_site')

import numpy as np
import ml_dtypes

import concourse.bass as bass
import concourse.tile as tile
from concourse import mybir
from concourse.vector_clock import ScopedClock
from concourse.bass_utils import run_bass_kernel_spmd

f32 = mybir.dt.float32
f32r = mybir.dt.float32r
bf16 = mybir.dt.bfloat16
AF = mybir.ActivationFunctionType
AX = mybir.AxisListType
ALU = mybir.AluOpType

N_CORES = 8
C = 512
N = 2048
HEADS = 8
D = 64
K_IDX = 128
GROUPS = 32
GSIZE = C // GROUPS          # 16 channels per group
NQ = N // N_CORES            # 256 queries per core
NCHUNK = N // 128            # 16 key chunks
CCHUNK = C // 128            # 4 channel chunks
EPS = 1e-6

# head -> block mapping: even heads (lhsT base partition 0) in even-bank
# score slots, odd heads in odd banks, so concurrently-issued row-group
# pairs never share a PSUM bank.
BLK = [4 * (h // 4) + (h % 4) // 2 + 2 * (h % 2) for h in range(HEADS)]
HB = [0] * 8
for _h in range(HEADS):
    HB[BLK[_h]] = _h                                     # b -> h

# ---------------------------------------------------------------------------
# walrus workaround: this container's walrus accepts at most ONE embedded
# sync-wait per engine instruction. Split Tile's multi-wait instructions
# into chains of single-wait NoOps, and do the same for the kernel-tail
# drain that Tile emits at TileContext exit.
# ---------------------------------------------------------------------------
_wsplit = [0]


def _drain_and_barrier_split(self, tick_clock, wait_clock):
    nc = self.nc
    carrier = nc.sync.nop(nofuse=True)
    wait_clock.add_sem_waits(
        carrier.ins, ScopedClock({None: tick_clock.global_clock}))
    si = carrier.ins.sync_info
    waits = list(si.on_wait or []) if si is not None else []
    if len(waits) > 1:
        carrier.ins.sync_info = mybir.SyncInfo(
            on_wait=waits[:1], on_update=list(si.on_update or []))
        for w in waits[1:]:
            extra = nc.sync.nop(nofuse=True)
            extra.ins.sync_info = mybir.SyncInfo(on_wait=[w], on_update=[])
    nc.sync.drain()
    nc.all_engine_barrier(sem_only=True)
    assert self.sems is not None
    popped = nc._tile_sem_poison_stack.pop()
    assert popped is self._sem_poison
    nc.clear_and_free_semaphores(list(self.sems.allocated().values()))
    nc.all_engine_barrier(sem_only=True)


def _split_sync_waits(nc, max_waits=1):
    for f in nc.m.functions:
        for bb in f.blocks:
            insts = bb.instructions
            out = []
            changed = False
            for inst in insts:
                si = inst.sync_info
                waits = list(si.on_wait or []) if si is not None else []
                if len(waits) > max_waits:
                    changed = True
                    for i in range(len(waits) - max_waits):
                        _wsplit[0] += 1
                        nop = mybir.InstNoOp(
                            name=f"I-wsplit-{_wsplit[0]}", ins=[], outs=[])
                        nop.engine = inst.engine
                        nop.sync_info = mybir.SyncInfo(
                            on_wait=[waits[i]], on_update=[])
                        out.append(nop)
                    inst.sync_info = mybir.SyncInfo(
                        on_wait=waits[len(waits) - max_waits:],
                        on_update=list(si.on_update or []))
                out.append(inst)
            if changed:
                if isinstance(insts, list):
                    insts[:] = out
                else:
                    bb.instructions = out


tile.TileContext._drain_and_barrier = _drain_and_barrier_split


# ---------------------------------------------------------------------------
# kernel builder
# ---------------------------------------------------------------------------

def _build(split_waits=True):
    nc = bass.Bass("TRN2", target_bir_lowering=False, debug=False)

    def din(name, shape, dt=f32):
        return nc.dram_tensor(name, shape, dt, kind="ExternalInput").ap()

    xbf_d = din("xbf", [C, N], bf16)
    xq_d = din("xq", [C, NQ])
    cnt_d = din("cnt", [N, NQ], bf16)
    wkT_d = din("wkT", [C, C])
    wqT_d = din("wqT", [C, C])
    wvT_d = din("wvT", [C, C])
    wpTb_d = din("wpTb", [C, C])
    smalls_d = din("smalls", [128, 20])
    brow_d = din("brow", [2, C])
    gind_d = din("gind", [128, 32 * CCHUNK])
    gindT_d = din("gindT", [GROUPS, C])
    out_d = nc.dram_tensor("out", [C, NQ], f32, kind="ExternalOutput").ap()

    with tile.TileContext(nc) as tc, contextlib.ExitStack() as ctx:
        P = ctx.enter_context(tc.tile_pool(name="persist", bufs=1))
        # big shared pool: 4 x-chunk tiles + 16 persisted softmax-weight
        # pair tiles cycle through 16 buffers (the last 4 wt pairs reuse
        # the x space once the affine has consumed it)
        BIGP = ctx.enter_context(tc.tile_pool(name="bigp", bufs=16))
        A_cm = tc.tile_pool(name="phase_a", bufs=1)
        A = A_cm.__enter__()

        # ---- early DMAs: x first (GN stats are the critical path) ----
        xt = [BIGP.tile([128, N], bf16, tag="big", name=f"xt{k}")
              for k in range(CCHUNK)]
        for k in range(CCHUNK):
            nc.gpsimd.dma_start(xt[k][:], xbf_d[128 * k:128 * (k + 1), :])
        smallst = P.tile([128, 20], f32, tag="smalls", name="smalls")
        nc.sync.dma_start(smallst[:], smalls_d)
        gindt = P.tile([128, 32 * CCHUNK], f32, tag="gind", name="gind")
        nc.sync.dma_start(gindt[:], gind_d)
        gindTt = P.tile([GROUPS, C], f32, tag="gindT", name="gindT")
        nc.sync.dma_start(gindTt[:], gindT_d)
        bvrow_t = P.tile([1, C], f32, tag="bvrow", name="bvrow")
        nc.sync.dma_start(bvrow_t[:], brow_d[1:2, :])
        xqt = [P.tile([128, NQ], f32, tag=f"xqt{k}", name=f"xqt{k}")
               for k in range(CCHUNK)]
        for k in range(CCHUNK):
            nc.sync.dma_start(xqt[k][:], xq_d[128 * k:128 * (k + 1), :])

        def sm(k, f):
            return smallst[:, 5 * k + f:5 * k + f + 1]

        # ---- GroupNorm stats, pipelined per x chunk ----
        s1 = [P.tile([128, 1], f32, tag=f"s1{k}", name=f"s1{k}")
              for k in range(CCHUNK)]
        s2 = [P.tile([128, 1], f32, tag=f"s2{k}", name=f"s2{k}")
              for k in range(CCHUNK)]
        At = [P.tile([128, 1], f32, tag=f"A{k}", name=f"A{k}")
              for k in range(CCHUNK)]
        Bt = [P.tile([128, 1], f32, tag=f"B{k}", name=f"B{k}")
              for k in range(CCHUNK)]
        sq = BIGP.tile([128, N], bf16, tag="big", name="sq")
        epst = P.tile([GROUPS, 1], f32, tag="epst", name="epst")
        nc.gpsimd.memset(epst[:], float(EPS))
        with tc.tile_pool(name="gnps", bufs=1, space="PSUM") as gnps:
            for k in range(CCHUNK):
                nc.vector.tensor_reduce(s1[k][:], xt[k][:],
                                        axis=AX.X, op=ALU.add)
                nc.scalar.activation(sq[:], xt[k][:], AF.Square,
                                     accum_out=s2[k][:])
            gs = gnps.tile([GROUPS, 2], f32, tag="gs", name="gs")
            for k in range(CCHUNK):
                nc.tensor.matmul(gs[:, 0:1], gindt[:, 32 * k:32 * (k + 1)],
                                 s1[k][:], start=(k == 0),
                                 stop=(k == CCHUNK - 1))
            for k in range(CCHUNK):
                nc.tensor.matmul(gs[:, 1:2], gindt[:, 32 * k:32 * (k + 1)],
                                 s2[k][:], start=(k == 0),
                                 stop=(k == CCHUNK - 1))
            mstat = P.tile([GROUPS, 2], f32, tag="mstat", name="mstat")
            inv_n = 1.0 / (GSIZE * N)
            nc.vector.tensor_scalar_mul(mstat[:, 0:1], gs[:, 0:1], inv_n)
            m2 = P.tile([GROUPS, 1], f32, tag="m2", name="m2")
            nc.vector.tensor_mul(m2[:], mstat[:, 0:1], mstat[:, 0:1])
            var = P.tile([GROUPS, 1], f32, tag="var", name="var")
            nc.vector.scalar_tensor_tensor(var[:], gs[:, 1:2], inv_n, m2[:],
                                           op0=ALU.mult, op1=ALU.subtract)
            # 1/sqrt(var+eps) = exp(-0.5 ln(var+eps)); eps rides the Ln
            # bias, exp table set only
            lv = P.tile([GROUPS, 1], f32, tag="lv", name="lv")
            nc.scalar.activation(lv[:], var[:], AF.Ln, bias=epst[:, 0:1])
            nc.scalar.activation(mstat[:, 1:2], lv[:], AF.Exp, scale=-0.5)
            mr = [P.tile([128, 2], f32, tag=f"mr{k}", name=f"mr{k}")
                  for k in range(CCHUNK)]
            for k in range(CCHUNK):
                mrp = gnps.tile([128, 2], f32, tag="mrp", name="mrp", bufs=2)
                nc.tensor.matmul(mrp[:], gindTt[:, 128 * k:128 * (k + 1)],
                                 mstat[:], start=True, stop=True)
                nc.vector.tensor_copy(mr[k][:], mrp[:])
            for k in range(CCHUNK):
                nc.vector.tensor_mul(At[k][:], sm(k, 3), mr[k][:, 1:2])
                tmp = P.tile([128, 1], f32, tag="tmpB", name="tmpB")
                nc.vector.tensor_mul(tmp[:], mr[k][:, 0:1], At[k][:])
                nc.vector.tensor_sub(Bt[k][:], sm(k, 4), tmp[:])
        # ---- affine: hq (query slice, f32 in) then h (full, bf16 in) ----
        hqt = [P.tile([128, NQ], f32r, tag=f"hqt{k}", name=f"hqt{k}")
               for k in range(CCHUNK)]
        ht = [A.tile([128, N], f32r, tag=f"ht{k}", name=f"ht{k}")
              for k in range(CCHUNK)]
        with nc.allow_low_precision(reason="f32r affine"):
            for k in range(CCHUNK):
                nc.vector.tensor_scalar(
                    hqt[k][:], xqt[k][:], At[k][:, 0:1], Bt[k][:, 0:1],
                    op0=ALU.mult, op1=ALU.add)
            for g in range(4):
                cols = slice(512 * g, 512 * (g + 1))
                for k in range(CCHUNK):
                    nc.vector.tensor_scalar(
                        ht[k][:, cols], xt[k][:, cols],
                        At[k][:, 0:1], Bt[k][:, 0:1],
                        op0=ALU.mult, op1=ALU.add)

        # ---- weight DMAs (behind x in queue order) ----
        wq = [A.tile([128, C], f32r, tag=f"wq{k}", name=f"wq{k}")
              for k in range(CCHUNK)]
        wk = [A.tile([128, C], f32r, tag=f"wk{k}", name=f"wk{k}")
              for k in range(CCHUNK)]
        wv = [A.tile([128, C], f32r, tag=f"wv{k}", name=f"wv{k}")
              for k in range(CCHUNK)]
        for k in range(CCHUNK):
            r = slice(128 * k, 128 * (k + 1))
            nc.gpsimd.dma_start(wq[k][:], wqT_d[r, :])
        for k in range(CCHUNK):
            r = slice(128 * k, 128 * (k + 1))
            nc.gpsimd.dma_start(wk[k][:], wkT_d[r, :])
        for k in range(CCHUNK):
            r = slice(128 * k, 128 * (k + 1))
            nc.gpsimd.dma_start(wv[k][:], wvT_d[r, :])
        cntt = P.tile([128, 256 * NCHUNK], bf16, tag="cntt", name="cntt")
        nc.sync.dma_start(
            cntt[:].rearrange("p (m q) -> p m q", m=NCHUNK),
            cnt_d.rearrange("(m p) q -> p m q", p=128))

        kt = [P.tile([128, N], bf16, tag=f"kt{k}", name=f"kt{k}")
              for k in range(CCHUNK)]
        qt = [P.tile([128, NQ], bf16, tag=f"qt{k}", name=f"qt{k}")
              for k in range(CCHUNK)]
        vt = [P.tile([128, 65 * HEADS], bf16, tag=f"vt{m}", name=f"vt{m}")
              for m in range(NCHUNK)]
        on = P.tile([64, 256 * HEADS], bf16, tag="on", name="on")
        oraw = [P.tile([65, 256 * 4], bf16, tag=f"oraw{p}", name=f"oraw{p}")
                for p in range(2)]
        rr = [P.tile([1, 256 * 4], bf16, tag=f"rr{p}", name=f"rr{p}")
              for p in range(2)]
        rb = [P.tile([64, 256 * 4], bf16, tag=f"rb{p}", name=f"rb{p}")
              for p in range(2)]
        lnt = [P.tile([1, 256 * 4], f32, tag=f"lnt{p}", name=f"lnt{p}")
               for p in range(2)]
        onesr = P.tile([1, 128], f32, tag="onesr", name="onesr")
        nc.vector.memset(onesr[:], 1.0)
        onesb = P.tile([1, 64], bf16, tag="onesb", name="onesb")
        nc.gpsimd.memset(onesb[:], 1.0)
        nbias = P.tile([1, 1], f32, tag="nbias", name="nbias")
        nc.gpsimd.memset(nbias[:], float(-32.0 * np.log(2.0)))

        # ==== phase 1: convs fused with both passes' scores/exp/cnt ====
        # The score->exp->cnt chains ride along under the dense conv
        # matmul stream so the PE never idles (keeps HAM at 8/8); the
        # masked softmax weights for all 32 (pass, chunk) tiles persist
        # in SBUF until the OV streak in phase 2.
        wtpair = {}          # (p, m//2) -> [128, 2048] bf16 pair tile

        def wt_slice(p, m):
            key = (p, m // 2)
            if key not in wtpair:
                wtpair[key] = BIGP.tile([128, N], bf16, tag="big",
                                        name=f"wt{p}_{m // 2}")
            half = m % 2
            return wtpair[key][:, 1024 * half:1024 * (half + 1)]

        with tc.tile_pool(name="asb", bufs=2) as asb, \
                tc.tile_pool(name="cps", bufs=4, space="PSUM") as cps, \
                tc.tile_pool(name="sps", bufs=2, space="PSUM") as sps:

            # Q conv; K bias is softmax-invariant and dropped entirely.
            for m in range(CCHUNK):
                pq = cps.tile([128, 512], f32, tag="cp", name="cpq")[:, 0:NQ]
                for ci in range(CCHUNK):
                    nc.tensor.matmul(pq[:], wq[ci][:, 128 * m:128 * (m + 1)],
                                     hqt[ci][:],
                                     start=(ci == 0), stop=(ci == CCHUNK - 1))
                nc.scalar.activation(qt[m][:], pq[:], AF.Identity,
                                     bias=sm(m, 1))

            # bv broadcast [128, C] once
            bvb = P.tile([128, C], f32, tag="bvb", name="bvb")
            pbv2 = cps.tile([128, C], f32, tag="cp", name="cpbv2")
            nc.tensor.matmul(pbv2[:], onesr[0:1, 0:128], bvrow_t[0:1, :],
                             start=True, stop=True)
            nc.vector.tensor_copy(bvb[:], pbv2[:])

            def kconv_colgroup(j):
                cols = slice(512 * j, 512 * (j + 1))
                for m in range(CCHUNK):
                    pk = cps.tile([128, 512], f32, tag="cp", name="cpk")
                    for ci in range(CCHUNK):
                        nc.tensor.matmul(
                            pk[:], wk[ci][:, 128 * m:128 * (m + 1)],
                            ht[ci][:, cols],
                            start=(ci == 0), stop=(ci == CCHUNK - 1))
                    nc.vector.tensor_copy(kt[m][:, cols], pk[:])

            def vconv(m):
                pv = cps.tile([128, C], f32, tag="cp", name="cpv")
                for ci in range(CCHUNK):
                    nc.tensor.matmul(pv[:],
                                     ht[ci][:, 128 * m:128 * (m + 1)],
                                     wv[ci][:], start=(ci == 0),
                                     stop=(ci == CCHUNK - 1))
                dst = vt[m][:].rearrange("p (h e) -> p h e", h=HEADS)[:, :, 0:64]
                nc.vector.scalar_tensor_tensor(
                    dst, pv[:].rearrange("p (h d) -> p h d", h=HEADS), 1.0,
                    bvb[:].rearrange("p (h d) -> p h d", h=HEADS),
                    op0=ALU.mult, op1=ALU.add)
                ones_cols = vt[m][:].rearrange(
                    "p (h e) -> p h e", h=HEADS)[:, :, 64:65]
                nc.gpsimd.memset(ones_cols, 1.0)

            def attn_scores(p, m):
                heads = range(4 * p, 4 * p + 4)
                st = sps.tile([128, 256 * 4], f32, tag="st", name=f"st{p}_{m}")
                for h in heads:
                    par = h % 2
                    cm = h // 2
                    lb = BLK[h] - 4 * p
                    nc.tensor.matmul(
                        st[:, 256 * lb:256 * (lb + 1)],
                        kt[cm][64 * par:64 * (par + 1),
                               128 * m:128 * (m + 1)],
                        qt[cm][64 * par:64 * (par + 1), :],
                        start=True, stop=True)
                et = asb.tile([128, 256 * 4], bf16, tag="et", name=f"et{p}_{m}")
                nc.scalar.activation(et[:], st[:], AF.Exp)
                eng = nc.vector if (m + p) % 2 == 0 else nc.gpsimd
                eng.tensor_mul(
                    wt_slice(p, m).rearrange("p (b q) -> p b q", b=4),
                    et[:].rearrange("p (b q) -> p b q", b=4),
                    cntt[:, 256 * m:256 * (m + 1)].unsqueeze(1)
                        .broadcast_to([128, 4, NQ]))

            # fused burst: conv col-group, V chunks, then scores for the
            # freshly available key chunks (both passes)
            for j in range(4):
                kconv_colgroup(j)
                for m in range(4 * j, 4 * j + 4):
                    vconv(m)
                for m in range(4 * j, 4 * j + 4):
                    attn_scores(0, m)
                    attn_scores(1, m)

        A_cm.__exit__(None, None, None)   # ht + conv weights now dead

        # ==== phase 2: pure OV matmul streak + overlapped norms ====
        def attn_ov(p, m, ot):
            wt = wt_slice(p, m)
            for h in range(4 * p, 4 * p + 4):
                lb = BLK[h] - 4 * p
                nc.tensor.matmul(
                    ot[0:65, 512 * lb:512 * lb + 256],
                    vt[m][:, 65 * h:65 * h + 65],
                    wt[:, 256 * lb:256 * (lb + 1)],
                    start=(m == 0), stop=(m == NCHUNK - 1))

        def norm_copy(p, ot):
            # free the PSUM accumulator fast: strided copies -> SBUF
            # bf16, split across DVE and ACT so the bubble halves
            src = ot[0:65, :].rearrange("p (b w q) -> p b w q",
                                        b=4, w=2)[:, :, 0, :]
            dst = oraw[p][:].rearrange("p (b q) -> p b q", b=4)
            nc.vector.tensor_copy(dst[:, 0:2], src[:, 0:2])
            nc.scalar.activation(dst[:, 2:4], src[:, 2:4], AF.Identity)

        with tc.tile_pool(name="ops", bufs=1, space="PSUM") as ops, \
                tc.tile_pool(name="pps", bufs=2, space="PSUM") as pps, \
                tc.tile_pool(name="psb", bufs=2) as psb:
            wpb = [psb.tile([64, C], bf16, tag=f"wpb{b}", name=f"wpb{b}",
                            bufs=1)
                   for b in range(HEADS)]
            for b in range(HEADS):
                nc.gpsimd.dma_start(wpb[b][:], wpTb_d[64 * b:64 * (b + 1), :])

            def norm_math(p, j):
                # 1/s = exp(-ln s); ACT Ln is only accurate for
                # |ln x| < ~40, so pre-scale by 2^-32 (free activation
                # scale) and compensate in the Exp bias:
                # 1/s = exp(-ln(s 2^-32) - 32 ln 2). Broadcast across the
                # 64 d-partitions via a rank-1 PE matmul. j = 512-col
                # half (2 head blocks), so downstream proj can start
                # after the first half.
                cols = slice(512 * j, 512 * (j + 1))
                nc.scalar.activation(lnt[p][:, cols], oraw[p][64:65, cols],
                                     AF.Ln, scale=float(2.0 ** -32))
                nc.scalar.activation(rr[p][:, cols], lnt[p][:, cols],
                                     AF.Exp, scale=-1.0, bias=nbias[:, 0:1])
                rbp = pps.tile([64, 512], f32, tag="rbp",
                               name=f"rbp{p}_{j}", bufs=1)
                nc.tensor.matmul(rbp[:], onesb[:], rr[p][:, cols],
                                 start=True, stop=True)
                nc.vector.tensor_copy(rb[p][:, cols], rbp[:])
                nc.vector.tensor_mul(
                    on[:, 1024 * p + 512 * j:1024 * p + 512 * (j + 1)],
                    oraw[p][0:64, cols], rb[p][:, cols])

            ot0 = ops.tile([65, 512 * 4], f32, tag="ot", name="ot0")
            for m in range(NCHUNK):
                attn_ov(0, m, ot0)
            norm_copy(0, ot0)
            ot1 = ops.tile([65, 512 * 4], f32, tag="ot", name="ot1")
            for m in range(NCHUNK):
                attn_ov(1, m, ot1)
                if m == 1:
                    norm_math(0, 0)
                elif m == 3:
                    norm_math(0, 1)
            norm_copy(1, ot1)

            # proj m0..m2 pass-0 blocks fill the PE while norm-1 runs
            pj3 = []
            for m in range(3):
                pjm = pps.tile([128, NQ], f32, tag="pj", name=f"pj{m}",
                               bufs=3)
                pj3.append(pjm)
                for b in range(4):
                    nc.tensor.matmul(pjm[:],
                                     wpb[b][:, 128 * m:128 * (m + 1)],
                                     on[:, 256 * b:256 * (b + 1)],
                                     start=(b == 0), stop=False)
            norm_math(1, 0)
            for m in range(3):
                for b in range(4, 6):
                    nc.tensor.matmul(pj3[m][:],
                                     wpb[b][:, 128 * m:128 * (m + 1)],
                                     on[:, 256 * b:256 * (b + 1)],
                                     start=False, stop=False)
            norm_math(1, 1)

            def proj_epilogue(m, pjm, b0):
                for b in range(b0, HEADS):
                    nc.tensor.matmul(pjm[:],
                                     wpb[b][:, 128 * m:128 * (m + 1)],
                                     on[:, 256 * b:256 * (b + 1)],
                                     start=(b == 0), stop=(b == HEADS - 1))
                t1 = psb.tile([128, NQ], f32, tag="t1", name=f"t1{m}")
                nc.scalar.activation(t1[:], pjm[:], AF.Identity,
                                     bias=sm(m, 2))
                nc.vector.tensor_add(xqt[m][:], t1[:], xqt[m][:])
                nc.sync.dma_start(out_d[128 * m:128 * (m + 1), :], xqt[m][:])

            for m in range(3):
                proj_epilogue(m, pj3[m], 6)
            pjm = pps.tile([128, NQ], f32, tag="pj", name="pj3", bufs=3)
            proj_epilogue(3, pjm, 0)

    if split_waits:
        _split_sync_waits(nc)
    return nc


# ---------------------------------------------------------------------------
# host-side input prep + entry point
# ---------------------------------------------------------------------------

def _prep_inputs(x, valid_indices_mask, attendable_indices, gn_w, gn_b,
                 wq_, bq_, wk_, bk_, wv_, bv_, wp_, bp_):
    x = np.asarray(x, np.float32).reshape(C, N)
    idx = np.asarray(attendable_indices, np.int64)
    val = np.asarray(valid_indices_mask, np.float32)
    cnt_qn = np.zeros((N, N), np.float32)       # [q, n]
    rows = np.repeat(np.arange(N), K_IDX)
    np.add.at(cnt_qn, (rows, idx.reshape(-1)), val.reshape(-1))
    cntT = np.ascontiguousarray(cnt_qn.T).astype(ml_dtypes.bfloat16)  # [n, q]

    wq_ = np.asarray(wq_, np.float32)
    wk_ = np.asarray(wk_, np.float32)
    wv_ = np.asarray(wv_, np.float32)
    wp_ = np.asarray(wp_, np.float32)
    # wp column for o-channel (d*HEADS + h); our block order stacks head
    # HB[b] rows d-major at 64*b
    wpT = wp_.T                                    # [cin = d*8+h, cout]
    wpTb = np.empty((C, C), np.float32)
    for b in range(HEADS):
        h = HB[b]
        wpTb[64 * b:64 * (b + 1), :] = wpT[h::HEADS, :]   # d-major rows of head h

    gind = np.zeros((C, GROUPS), np.float32)
    gind[np.arange(C), np.arange(C) // GSIZE] = 1.0

    smalls = np.zeros((128, 20), np.float32)
    fields = [np.asarray(bk_, np.float32), np.asarray(bq_, np.float32),
              np.asarray(bp_, np.float32), np.asarray(gn_w, np.float32),
              np.asarray(gn_b, np.float32)]
    for k in range(CCHUNK):
        for f, arr in enumerate(fields):
            smalls[:, 5 * k + f] = arr.reshape(C)[128 * k:128 * (k + 1)]
    gind_all = np.zeros((128, 32 * CCHUNK), np.float32)
    for k in range(CCHUNK):
        gind_all[:, 32 * k:32 * (k + 1)] = gind[128 * k:128 * (k + 1), :]
    brow = np.stack([np.asarray(bq_, np.float32).reshape(C),
                     np.asarray(bv_, np.float32).reshape(C)])
    common = {
        "xbf": x.astype(ml_dtypes.bfloat16),
        "wkT": np.ascontiguousarray(wk_.T),
        "wqT": np.ascontiguousarray(wq_.T),
        "wvT": np.ascontiguousarray(wv_.T),
        "wpTb": wpTb,
        "smalls": smalls,
        "brow": brow,
        "gind": gind_all,
        "gindT": np.ascontiguousarray(gind.T),
    }
    in_maps = []
    for c in range(N_CORES):
        cols = slice(NQ * c, NQ * (c + 1))
        m = dict(common)
        m["xq"] = np.ascontiguousarray(x[:, cols])
        m["cnt"] = np.ascontiguousarray(cntT[:, cols])
        in_maps.append(m)
    return in_maps


def _enable_profile_hook():
    """Register the axon NTFF hook (this container's antenv lacks it)."""
    import antenv
    if 'antenv.axon_hooks' not in sys.modules:
        mod = types.ModuleType('antenv.axon_hooks')
        mod._hook = None
        mod.set_axon_ntff_profile_hook = lambda h: setattr(mod, '_hook', h)
        mod.get_axon_ntff_profile_hook = lambda: mod._hook
        sys.modules['antenv.axon_hooks'] = mod
        antenv.axon_hooks = mod
    from trn_agent_boot.trn_boot import _ntff_profile_via_ctypes
    sys.modules['antenv.axon_hooks'].set_axon_ntff_profile_hook(
        _ntff_profile_via_ctypes('/opt/axon/libaxon_pjrt.so'))
    import concourse.bass_utils as bu
    bu.upload_artifacts = lambda tmpdir: tmpdir


_CACHE = {}


def _run(inputs, trace=False):
    if "nc" not in _CACHE:
        _CACHE["nc"] = _build()
    nc = _CACHE["nc"]
    in_maps = _prep_inputs(
        inputs['x'], inputs['valid_indices_mask'],
        inputs['attendable_indices'], inputs['gn_w'], inputs['gn_b'],
        inputs['wq'], inputs['bq'], inputs['wk'], inputs['bk'],
        inputs['wv'], inputs['bv'], inputs['wp'], inputs['bp'])
    if trace:
        _enable_profile_hook()
    res = run_bass_kernel_spmd(nc, in_maps, list(range(N_CORES)), trace=trace)
    out = np.concatenate([res.results[c]["out"] for c in range(N_CORES)],
                         axis=1).reshape(1, C, N).astype(np.float32)
    return out, res


def kernel(**inputs):
    out, _ = _run(inputs, trace=False)
    return out


# revision 55
# speedup vs baseline: 1.1160x; 1.1160x over previous
"""Trainium2 Bass kernel for nn_AttnBlock_16887811407979 (sparse attention).

Strategy: 8-way sequence-parallel SPMD (each core handles a 256-query
slice, all heads), no collectives. The sparse gather is densified: the
host converts (attendable_indices, valid_indices_mask) into a dense
count matrix C[n, q], so softmax-over-slots == count-weighted dense
softmax:
    W[n,q] = C[n,q] * exp(S^T[n,q]);  O[q] = (W^T V) / sum_n W[n,q].

Orchestration (vs the original 147us version):
  - x shipped bf16 (halves the critical head DMA); kt/qt/vt/wt bf16
  - single ACT table set (natural_log_exp): GN 1/sqrt(var+eps) via
    exp(-0.5 ln(var+eps)), softmax 1/s via exp(-ln(s 2^-32) - 32 ln 2)
    (the Ln LUT is only accurate for |ln x| < ~40) -- kills the sqrt
    table load and the 4.4us single-partition DVE reciprocals
  - K bias dropped (softmax-invariant)
  - phase 1 fuses the K/V conv matmul stream with BOTH passes'
    score->exp->cnt-mul chains, so the PE never idles long enough for
    HAM to re-throttle; the 32 masked-weight tiles persist in SBUF
    (sharing buffers with the consumed x tiles)
  - phase 2 is a pure OV matmul streak; O^T accumulators are copied
    out of PSUM immediately (bf16) and softmax normalization runs on
    SBUF overlapped with the following matmuls; proj head-blocks
    accumulate while the pass-1 normalization finishes
"""
import sys
import types
import contextlib

sys.path.insert(0, '/opt/trn_rl_repo')
sys.path.insert(0, '/root/.axon_site')

import numpy as np
import ml_dtypes

import concourse.bass as bass
import concourse.tile as tile
from concourse import mybir
from concourse.vector_clock import ScopedClock
from concourse.bass_utils import run_bass_kernel_spmd

f32 = mybir.dt.float32
f32r = mybir.dt.float32r
bf16 = mybir.dt.bfloat16
AF = mybir.ActivationFunctionType
AX = mybir.AxisListType
ALU = mybir.AluOpType

N_CORES = 8
C = 512
N = 2048
HEADS = 8
D = 64
K_IDX = 128
GROUPS = 32
GSIZE = C // GROUPS          # 16 channels per group
NQ = N // N_CORES            # 256 queries per core
NCHUNK = N // 128            # 16 key chunks
CCHUNK = C // 128            # 4 channel chunks
EPS = 1e-6

# head -> block mapping: even heads (lhsT base partition 0) in even-bank
# score slots, odd heads in odd banks, so concurrently-issued row-group
# pairs never share a PSUM bank.
BLK = [4 * (h // 4) + (h % 4) // 2 + 2 * (h % 2) for h in range(HEADS)]
HB = [0] * 8
for _h in range(HEADS):
    HB[BLK[_h]] = _h                                     # b -> h

# ---------------------------------------------------------------------------
# walrus workaround: this container's walrus accepts at most ONE embedded
# sync-wait per engine instruction. Split Tile's multi-wait instructions
# into chains of single-wait NoOps, and do the same for the kernel-tail
# drain that Tile emits at TileContext exit.
# ---------------------------------------------------------------------------
_wsplit = [0]


def _drain_and_barrier_split(self, tick_clock, wait_clock):
    nc = self.nc
    carrier = nc.sync.nop(nofuse=True)
    wait_clock.add_sem_waits(
        carrier.ins, ScopedClock({None: tick_clock.global_clock}))
    si = carrier.ins.sync_info
    waits = list(si.on_wait or []) if si is not None else []
    if len(waits) > 1:
        carrier.ins.sync_info = mybir.SyncInfo(
            on_wait=waits[:1], on_update=list(si.on_update or []))
        for w in waits[1:]:
            extra = nc.sync.nop(nofuse=True)
            extra.ins.sync_info = mybir.SyncInfo(on_wait=[w], on_update=[])
    nc.sync.drain()
    nc.all_engine_barrier(sem_only=True)
    assert self.sems is not None
    popped = nc._tile_sem_poison_stack.pop()
    assert popped is self._sem_poison
    nc.clear_and_free_semaphores(list(self.sems.allocated().values()))
    nc.all_engine_barrier(sem_only=True)


def _split_sync_waits(nc, max_waits=1):
    for f in nc.m.functions:
        for bb in f.blocks:
            insts = bb.instructions
            out = []
            changed = False
            for inst in insts:
                si = inst.sync_info
                waits = list(si.on_wait or []) if si is not None else []
                if len(waits) > max_waits:
                    changed = True
                    for i in range(len(waits) - max_waits):
                        _wsplit[0] += 1
                        nop = mybir.InstNoOp(
                            name=f"I-wsplit-{_wsplit[0]}", ins=[], outs=[])
                        nop.engine = inst.engine
                        nop.sync_info = mybir.SyncInfo(
                            on_wait=[waits[i]], on_update=[])
                        out.append(nop)
                    inst.sync_info = mybir.SyncInfo(
                        on_wait=waits[len(waits) - max_waits:],
                        on_update=list(si.on_update or []))
                out.append(inst)
            if changed:
                if isinstance(insts, list):
                    insts[:] = out
                else:
                    bb.instructions = out


tile.TileContext._drain_and_barrier = _drain_and_barrier_split


# ---------------------------------------------------------------------------
# kernel builder
# ---------------------------------------------------------------------------

def _build(split_waits=True):
    nc = bass.Bass("TRN2", target_bir_lowering=False, debug=False)

    def din(name, shape, dt=f32):
        return nc.dram_tensor(name, shape, dt, kind="ExternalInput").ap()

    xbf_d = din("xbf", [C, N], bf16)
    xq_d = din("xq", [C, NQ])
    cnt_d = din("cnt", [N, NQ], bf16)
    wkT_d = din("wkT", [C, C])
    wqT_d = din("wqT", [C, C])
    wvT_d = din("wvT", [C, C])
    wpTb_d = din("wpTb", [C, C])
    smalls_d = din("smalls", [128, 20])
    brow_d = din("brow", [2, C])
    gind_d = din("gind", [128, 32 * CCHUNK])
    gindT_d = din("gindT", [GROUPS, C])
    out_d = nc.dram_tensor("out", [C, NQ], f32, kind="ExternalOutput").ap()

    with tile.TileContext(nc) as tc, contextlib.ExitStack() as ctx:
        P = ctx.enter_context(tc.tile_pool(name="persist", bufs=1))
        # big shared pool: 4 x-chunk tiles + 16 persisted softmax-weight
        # pair tiles cycle through 16 buffers (the last 4 wt pairs reuse
        # the x space once the affine has consumed it)
        BIGP = ctx.enter_context(tc.tile_pool(name="bigp", bufs=16))
        A_cm = tc.tile_pool(name="phase_a", bufs=1)
        A = A_cm.__enter__()

        # ---- early DMAs: x first (GN stats are the critical path) ----
        xt = [BIGP.tile([128, N], bf16, tag="big", name=f"xt{k}")
              for k in range(CCHUNK)]
        for k in range(CCHUNK):
            nc.gpsimd.dma_start(xt[k][:], xbf_d[128 * k:128 * (k + 1), :])
        smallst = P.tile([128, 20], f32, tag="smalls", name="smalls")
        nc.sync.dma_start(smallst[:], smalls_d)
        gindt = P.tile([128, 32 * CCHUNK], f32, tag="gind", name="gind")
        nc.sync.dma_start(gindt[:], gind_d)
        gindTt = P.tile([GROUPS, C], f32, tag="gindT", name="gindT")
        nc.sync.dma_start(gindTt[:], gindT_d)
        bvrow_t = P.tile([1, C], f32, tag="bvrow", name="bvrow")
        nc.sync.dma_start(bvrow_t[:], brow_d[1:2, :])
        xqt = [P.tile([128, NQ], f32, tag=f"xqt{k}", name=f"xqt{k}")
               for k in range(CCHUNK)]
        for k in range(CCHUNK):
            nc.sync.dma_start(xqt[k][:], xq_d[128 * k:128 * (k + 1), :])

        def sm(k, f):
            return smallst[:, 5 * k + f:5 * k + f + 1]

        # ---- GroupNorm stats, pipelined per x chunk ----
        s1 = [P.tile([128, 1], f32, tag=f"s1{k}", name=f"s1{k}")
              for k in range(CCHUNK)]
        s2 = [P.tile([128, 1], f32, tag=f"s2{k}", name=f"s2{k}")
              for k in range(CCHUNK)]
        At = [P.tile([128, 1], f32, tag=f"A{k}", name=f"A{k}")
              for k in range(CCHUNK)]
        Bt = [P.tile([128, 1], f32, tag=f"B{k}", name=f"B{k}")
              for k in range(CCHUNK)]
        sq = BIGP.tile([128, N], bf16, tag="big", name="sq")
        epst = P.tile([GROUPS, 1], f32, tag="epst", name="epst")
        nc.gpsimd.memset(epst[:], float(EPS))
        with tc.tile_pool(name="gnps", bufs=1, space="PSUM") as gnps:
            for k in range(CCHUNK):
                nc.vector.tensor_reduce(s1[k][:], xt[k][:],
                                        axis=AX.X, op=ALU.add)
                nc.scalar.activation(sq[:], xt[k][:], AF.Square,
                                     accum_out=s2[k][:])
            gs = gnps.tile([GROUPS, 2], f32, tag="gs", name="gs")
            for k in range(CCHUNK):
                nc.tensor.matmul(gs[:, 0:1], gindt[:, 32 * k:32 * (k + 1)],
                                 s1[k][:], start=(k == 0),
                                 stop=(k == CCHUNK - 1))
            for k in range(CCHUNK):
                nc.tensor.matmul(gs[:, 1:2], gindt[:, 32 * k:32 * (k + 1)],
                                 s2[k][:], start=(k == 0),
                                 stop=(k == CCHUNK - 1))
            mstat = P.tile([GROUPS, 2], f32, tag="mstat", name="mstat")
            inv_n = 1.0 / (GSIZE * N)
            nc.vector.tensor_scalar_mul(mstat[:, 0:1], gs[:, 0:1], inv_n)
            m2 = P.tile([GROUPS, 1], f32, tag="m2", name="m2")
            nc.vector.tensor_mul(m2[:], mstat[:, 0:1], mstat[:, 0:1])
            var = P.tile([GROUPS, 1], f32, tag="var", name="var")
            nc.vector.scalar_tensor_tensor(var[:], gs[:, 1:2], inv_n, m2[:],
                                           op0=ALU.mult, op1=ALU.subtract)
            # 1/sqrt(var+eps) = exp(-0.5 ln(var+eps)); eps rides the Ln
            # bias, exp table set only
            lv = P.tile([GROUPS, 1], f32, tag="lv", name="lv")
            nc.scalar.activation(lv[:], var[:], AF.Ln, bias=epst[:, 0:1])
            nc.scalar.activation(mstat[:, 1:2], lv[:], AF.Exp, scale=-0.5)
            mr = [P.tile([128, 2], f32, tag=f"mr{k}", name=f"mr{k}")
                  for k in range(CCHUNK)]
            for k in range(CCHUNK):
                mrp = gnps.tile([128, 2], f32, tag="mrp", name="mrp", bufs=2)
                nc.tensor.matmul(mrp[:], gindTt[:, 128 * k:128 * (k + 1)],
                                 mstat[:], start=True, stop=True)
                nc.vector.tensor_copy(mr[k][:], mrp[:])
            for k in range(CCHUNK):
                nc.vector.tensor_mul(At[k][:], sm(k, 3), mr[k][:, 1:2])
                tmp = P.tile([128, 1], f32, tag="tmpB", name="tmpB")
                nc.vector.tensor_mul(tmp[:], mr[k][:, 0:1], At[k][:])
                nc.vector.tensor_sub(Bt[k][:], sm(k, 4), tmp[:])
        # ---- affine: hq (query slice, f32 in) then h (full, bf16 in) ----
        hqt = [P.tile([128, NQ], f32r, tag=f"hqt{k}", name=f"hqt{k}")
               for k in range(CCHUNK)]
        ht = [A.tile([128, N], f32r, tag=f"ht{k}", name=f"ht{k}")
              for k in range(CCHUNK)]
        with nc.allow_low_precision(reason="f32r affine"):
            for k in range(CCHUNK):
                nc.vector.tensor_scalar(
                    hqt[k][:], xqt[k][:], At[k][:, 0:1], Bt[k][:, 0:1],
                    op0=ALU.mult, op1=ALU.add)
            for g in range(4):
                cols = slice(512 * g, 512 * (g + 1))
                for k in range(CCHUNK):
                    nc.vector.tensor_scalar(
                        ht[k][:, cols], xt[k][:, cols],
                        At[k][:, 0:1], Bt[k][:, 0:1],
                        op0=ALU.mult, op1=ALU.add)

        # ---- weight DMAs (behind x in queue order) ----
        wq = [A.tile([128, C], f32r, tag=f"wq{k}", name=f"wq{k}")
              for k in range(CCHUNK)]
        wk = [A.tile([128, C], f32r, tag=f"wk{k}", name=f"wk{k}")
              for k in range(CCHUNK)]
        wv = [A.tile([128, C], f32r, tag=f"wv{k}", name=f"wv{k}")
              for k in range(CCHUNK)]
        for k in range(CCHUNK):
            r = slice(128 * k, 128 * (k + 1))
            nc.gpsimd.dma_start(wq[k][:], wqT_d[r, :])
        for k in range(CCHUNK):
            r = slice(128 * k, 128 * (k + 1))
            nc.gpsimd.dma_start(wk[k][:], wkT_d[r, :])
        for k in range(CCHUNK):
            r = slice(128 * k, 128 * (k + 1))
            nc.gpsimd.dma_start(wv[k][:], wvT_d[r, :])
        cntt = P.tile([128, 256 * NCHUNK], bf16, tag="cntt", name="cntt")
        nc.sync.dma_start(
            cntt[:].rearrange("p (m q) -> p m q", m=NCHUNK),
            cnt_d.rearrange("(m p) q -> p m q", p=128))

        kt = [P.tile([128, N], bf16, tag=f"kt{k}", name=f"kt{k}")
              for k in range(CCHUNK)]
        qt = [P.tile([128, NQ], bf16, tag=f"qt{k}", name=f"qt{k}")
              for k in range(CCHUNK)]
        vt = [P.tile([128, 65 * HEADS], bf16, tag=f"vt{m}", name=f"vt{m}")
              for m in range(NCHUNK)]
        on = P.tile([64, 256 * HEADS], bf16, tag="on", name="on")
        oraw = [P.tile([65, 256 * 4], bf16, tag=f"oraw{p}", name=f"oraw{p}")
                for p in range(2)]
        rr = [P.tile([1, 256 * 4], bf16, tag=f"rr{p}", name=f"rr{p}")
              for p in range(2)]
        rb = [P.tile([64, 256 * 4], bf16, tag=f"rb{p}", name=f"rb{p}")
              for p in range(2)]
        lnt = [P.tile([1, 256 * 4], f32, tag=f"lnt{p}", name=f"lnt{p}")
               for p in range(2)]
        onesr = P.tile([1, 128], f32, tag="onesr", name="onesr")
        nc.vector.memset(onesr[:], 1.0)
        onesb = P.tile([1, 64], bf16, tag="onesb", name="onesb")
        nc.gpsimd.memset(onesb[:], 1.0)
        nbias = P.tile([1, 1], f32, tag="nbias", name="nbias")
        nc.gpsimd.memset(nbias[:], float(-32.0 * np.log(2.0)))

        # ==== phase 1: convs fused with both passes' scores/exp/cnt ====
        # The score->exp->cnt chains ride along under the dense conv
        # matmul stream so the PE never idles (keeps HAM at 8/8); the
        # masked softmax weights for all 32 (pass, chunk) tiles persist
        # in SBUF until the OV streak in phase 2.
        wtpair = {}          # (p, m//2) -> [128, 2048] bf16 pair tile

        def wt_slice(p, m):
            key = (p, m // 2)
            if key not in wtpair:
                wtpair[key] = BIGP.tile([128, N], bf16, tag="big",
                                        name=f"wt{p}_{m // 2}")
            half = m % 2
            return wtpair[key][:, 1024 * half:1024 * (half + 1)]

        with tc.tile_pool(name="asb", bufs=2) as asb, \
                tc.tile_pool(name="cps", bufs=4, space="PSUM") as cps, \
                tc.tile_pool(name="sps", bufs=2, space="PSUM") as sps:

            # Q conv; K bias is softmax-invariant and dropped entirely.
            for m in range(CCHUNK):
                pq = cps.tile([128, 512], f32, tag="cp", name="cpq")[:, 0:NQ]
                for ci in range(CCHUNK):
                    nc.tensor.matmul(pq[:], wq[ci][:, 128 * m:128 * (m + 1)],
                                     hqt[ci][:],
                                     start=(ci == 0), stop=(ci == CCHUNK - 1))
                nc.scalar.activation(qt[m][:], pq[:], AF.Identity,
                                     bias=sm(m, 1))

            # bv broadcast [128, C] once
            bvb = P.tile([128, C], f32, tag="bvb", name="bvb")
            pbv2 = cps.tile([128, C], f32, tag="cp", name="cpbv2")
            nc.tensor.matmul(pbv2[:], onesr[0:1, 0:128], bvrow_t[0:1, :],
                             start=True, stop=True)
            nc.vector.tensor_copy(bvb[:], pbv2[:])

            def kconv_colgroup(j):
                cols = slice(512 * j, 512 * (j + 1))
                for m in range(CCHUNK):
                    pk = cps.tile([128, 512], f32, tag="cp", name="cpk")
                    for ci in range(CCHUNK):
                        nc.tensor.matmul(
                            pk[:], wk[ci][:, 128 * m:128 * (m + 1)],
                            ht[ci][:, cols],
                            start=(ci == 0), stop=(ci == CCHUNK - 1))
                    nc.vector.tensor_copy(kt[m][:, cols], pk[:])

            def vconv(m):
                pv = cps.tile([128, C], f32, tag="cp", name="cpv")
                for ci in range(CCHUNK):
                    nc.tensor.matmul(pv[:],
                                     ht[ci][:, 128 * m:128 * (m + 1)],
                                     wv[ci][:], start=(ci == 0),
                                     stop=(ci == CCHUNK - 1))
                dst = vt[m][:].rearrange("p (h e) -> p h e", h=HEADS)[:, :, 0:64]
                nc.vector.scalar_tensor_tensor(
                    dst, pv[:].rearrange("p (h d) -> p h d", h=HEADS), 1.0,
                    bvb[:].rearrange("p (h d) -> p h d", h=HEADS),
                    op0=ALU.mult, op1=ALU.add)
                ones_cols = vt[m][:].rearrange(
                    "p (h e) -> p h e", h=HEADS)[:, :, 64:65]
                nc.gpsimd.memset(ones_cols, 1.0)

            def attn_scores(p, m):
                heads = range(4 * p, 4 * p + 4)
                st = sps.tile([128, 256 * 4], f32, tag="st", name=f"st{p}_{m}")
                for h in heads:
                    par = h % 2
                    cm = h // 2
                    lb = BLK[h] - 4 * p
                    nc.tensor.matmul(
                        st[:, 256 * lb:256 * (lb + 1)],
                        kt[cm][64 * par:64 * (par + 1),
                               128 * m:128 * (m + 1)],
                        qt[cm][64 * par:64 * (par + 1), :],
                        start=True, stop=True)
                et = asb.tile([128, 256 * 4], bf16, tag="et", name=f"et{p}_{m}")
                nc.scalar.activation(et[:], st[:], AF.Exp)
                nc.vector.tensor_mul(
                    wt_slice(p, m).rearrange("p (b q) -> p b q", b=4),
                    et[:].rearrange("p (b q) -> p b q", b=4),
                    cntt[:, 256 * m:256 * (m + 1)].unsqueeze(1)
                        .broadcast_to([128, 4, NQ]))

            # fused burst: conv col-group, V chunks, then scores for the
            # freshly available key chunks (both passes)
            for j in range(4):
                kconv_colgroup(j)
                for m in range(4 * j, 4 * j + 4):
                    vconv(m)
                for m in range(4 * j, 4 * j + 4):
                    attn_scores(0, m)
                    attn_scores(1, m)

        A_cm.__exit__(None, None, None)   # ht + conv weights now dead

        # ==== phase 2: pure OV matmul streak + overlapped norms ====
        def attn_ov(p, m, ot):
            wt = wt_slice(p, m)
            for h in range(4 * p, 4 * p + 4):
                lb = BLK[h] - 4 * p
                nc.tensor.matmul(
                    ot[0:65, 512 * lb:512 * lb + 256],
                    vt[m][:, 65 * h:65 * h + 65],
                    wt[:, 256 * lb:256 * (lb + 1)],
                    start=(m == 0), stop=(m == NCHUNK - 1))

        def norm_copy(p, ot):
            # free the PSUM accumulator fast: strided copies -> SBUF
            # bf16, split across DVE and ACT so the bubble halves
            src = ot[0:65, :].rearrange("p (b w q) -> p b w q",
                                        b=4, w=2)[:, :, 0, :]
            dst = oraw[p][:].rearrange("p (b q) -> p b q", b=4)
            nc.vector.tensor_copy(dst[:, 0:2], src[:, 0:2])
            nc.scalar.activation(dst[:, 2:4], src[:, 2:4], AF.Identity)

        with tc.tile_pool(name="ops", bufs=1, space="PSUM") as ops, \
                tc.tile_pool(name="pps", bufs=2, space="PSUM") as pps, \
                tc.tile_pool(name="psb", bufs=2) as psb:
            wpb = [psb.tile([64, C], bf16, tag=f"wpb{b}", name=f"wpb{b}",
                            bufs=1)
                   for b in range(HEADS)]
            for b in range(HEADS):
                nc.gpsimd.dma_start(wpb[b][:], wpTb_d[64 * b:64 * (b + 1), :])

            def norm_math(p, j):
                # 1/s = exp(-ln s); ACT Ln is only accurate for
                # |ln x| < ~40, so pre-scale by 2^-32 (free activation
                # scale) and compensate in the Exp bias:
                # 1/s = exp(-ln(s 2^-32) - 32 ln 2). Broadcast across the
                # 64 d-partitions via a rank-1 PE matmul. j = 512-col
                # half (2 head blocks), so downstream proj can start
                # after the first half.
                cols = slice(512 * j, 512 * (j + 1))
                nc.scalar.activation(lnt[p][:, cols], oraw[p][64:65, cols],
                                     AF.Ln, scale=float(2.0 ** -32))
                nc.scalar.activation(rr[p][:, cols], lnt[p][:, cols],
                                     AF.Exp, scale=-1.0, bias=nbias[:, 0:1])
                rbp = pps.tile([64, 512], f32, tag="rbp",
                               name=f"rbp{p}_{j}", bufs=1)
                nc.tensor.matmul(rbp[:], onesb[:], rr[p][:, cols],
                                 start=True, stop=True)
                nc.vector.tensor_copy(rb[p][:, cols], rbp[:])
                nc.vector.tensor_mul(
                    on[:, 1024 * p + 512 * j:1024 * p + 512 * (j + 1)],
                    oraw[p][0:64, cols], rb[p][:, cols])

            ot0 = ops.tile([65, 512 * 4], f32, tag="ot", name="ot0")
            for m in range(NCHUNK):
                attn_ov(0, m, ot0)
            norm_copy(0, ot0)
            ot1 = ops.tile([65, 512 * 4], f32, tag="ot", name="ot1")
            for m in range(NCHUNK):
                attn_ov(1, m, ot1)
                if m == 1:
                    norm_math(0, 0)
                elif m == 3:
                    norm_math(0, 1)
            norm_copy(1, ot1)

            # proj m0..m2 pass-0 blocks fill the PE while norm-1 runs
            pj3 = []
            for m in range(3):
                pjm = pps.tile([128, NQ], f32, tag="pj", name=f"pj{m}",
                               bufs=3)
                pj3.append(pjm)
                for b in range(4):
                    nc.tensor.matmul(pjm[:],
                                     wpb[b][:, 128 * m:128 * (m + 1)],
                                     on[:, 256 * b:256 * (b + 1)],
                                     start=(b == 0), stop=False)
            norm_math(1, 0)
            for m in range(3):
                for b in range(4, 6):
                    nc.tensor.matmul(pj3[m][:],
                                     wpb[b][:, 128 * m:128 * (m + 1)],
                                     on[:, 256 * b:256 * (b + 1)],
                                     start=False, stop=False)
            norm_math(1, 1)

            def proj_epilogue(m, pjm, b0):
                for b in range(b0, HEADS):
                    nc.tensor.matmul(pjm[:],
                                     wpb[b][:, 128 * m:128 * (m + 1)],
                                     on[:, 256 * b:256 * (b + 1)],
                                     start=(b == 0), stop=(b == HEADS - 1))
                t1 = psb.tile([128, NQ], f32, tag="t1", name=f"t1{m}")
                nc.scalar.activation(t1[:], pjm[:], AF.Identity,
                                     bias=sm(m, 2))
                nc.vector.tensor_add(xqt[m][:], t1[:], xqt[m][:])
                nc.sync.dma_start(out_d[128 * m:128 * (m + 1), :], xqt[m][:])

            for m in range(3):
                proj_epilogue(m, pj3[m], 6)
            pjm = pps.tile([128, NQ], f32, tag="pj", name="pj3", bufs=3)
            proj_epilogue(3, pjm, 0)

    if split_waits:
        _split_sync_waits(nc)
    return nc


# ---------------------------------------------------------------------------
# host-side input prep + entry point
# ---------------------------------------------------------------------------

def _prep_inputs(x, valid_indices_mask, attendable_indices, gn_w, gn_b,
                 wq_, bq_, wk_, bk_, wv_, bv_, wp_, bp_):
    x = np.asarray(x, np.float32).reshape(C, N)
    idx = np.asarray(attendable_indices, np.int64)
    val = np.asarray(valid_indices_mask, np.float32)
    cnt_qn = np.zeros((N, N), np.float32)       # [q, n]
    rows = np.repeat(np.arange(N), K_IDX)
    np.add.at(cnt_qn, (rows, idx.reshape(-1)), val.reshape(-1))
    cntT = np.ascontiguousarray(cnt_qn.T).astype(ml_dtypes.bfloat16)  # [n, q]

    wq_ = np.asarray(wq_, np.float32)
    wk_ = np.asarray(wk_, np.float32)
    wv_ = np.asarray(wv_, np.float32)
    wp_ = np.asarray(wp_, np.float32)
    # wp column for o-channel (d*HEADS + h); our block order stacks head
    # HB[b] rows d-major at 64*b
    wpT = wp_.T                                    # [cin = d*8+h, cout]
    wpTb = np.empty((C, C), np.float32)
    for b in range(HEADS):
        h = HB[b]
        wpTb[64 * b:64 * (b + 1), :] = wpT[h::HEADS, :]   # d-major rows of head h

    gind = np.zeros((C, GROUPS), np.float32)
    gind[np.arange(C), np.arange(C) // GSIZE] = 1.0

    smalls = np.zeros((128, 20), np.float32)
    fields = [np.asarray(bk_, np.float32), np.asarray(bq_, np.float32),
              np.asarray(bp_, np.float32), np.asarray(gn_w, np.float32),
              np.asarray(gn_b, np.float32)]
    for k in range(CCHUNK):
        for f, arr in enumerate(fields):
            smalls[:, 5 * k + f] = arr.reshape(C)[128 * k:128 * (k + 1)]
    gind_all = np.zeros((128, 32 * CCHUNK), np.float32)
    for k in range(CCHUNK):
        gind_all[:, 32 * k:32 * (k + 1)] = gind[128 * k:128 * (k + 1), :]
    brow = np.stack([np.asarray(bq_, np.float32).reshape(C),
                     np.asarray(bv_, np.float32).reshape(C)])
    common = {
        "xbf": x.astype(ml_dtypes.bfloat16),
        "wkT": np.ascontiguousarray(wk_.T),
        "wqT": np.ascontiguousarray(wq_.T),
        "wvT": np.ascontiguousarray(wv_.T),
        "wpTb": wpTb,
        "smalls": smalls,
        "brow": brow,
        "gind": gind_all,
        "gindT": np.ascontiguousarray(gind.T),
    }
    in_maps = []
    for c in range(N_CORES):
        cols = slice(NQ * c, NQ * (c + 1))
        m = dict(common)
        m["xq"] = np.ascontiguousarray(x[:, cols])
        m["cnt"] = np.ascontiguousarray(cntT[:, cols])
        in_maps.append(m)
    return in_maps


def _enable_profile_hook():
    """Register the axon NTFF hook (this container's antenv lacks it)."""
    import antenv
    if 'antenv.axon_hooks' not in sys.modules:
        mod = types.ModuleType('antenv.axon_hooks')
        mod._hook = None
        mod.set_axon_ntff_profile_hook = lambda h: setattr(mod, '_hook', h)
        mod.get_axon_ntff_profile_hook = lambda: mod._hook
        sys.modules['antenv.axon_hooks'] = mod
        antenv.axon_hooks = mod
    from trn_agent_boot.trn_boot import _ntff_profile_via_ctypes
    sys.modules['antenv.axon_hooks'].set_axon_ntff_profile_hook(
        _ntff_profile_via_ctypes('/opt/axon/libaxon_pjrt.so'))
    import concourse.bass_utils as bu
    bu.upload_artifacts = lambda tmpdir: tmpdir


_CACHE = {}


def _run(inputs, trace=False):
    if "nc" not in _CACHE:
        _CACHE["nc"] = _build()
    nc = _CACHE["nc"]
    in_maps = _prep_inputs(
        inputs['x'], inputs['valid_indices_mask'],
        inputs['attendable_indices'], inputs['gn_w'], inputs['gn_b'],
        inputs['wq'], inputs['bq'], inputs['wk'], inputs['bk'],
        inputs['wv'], inputs['bv'], inputs['wp'], inputs['bp'])
    if trace:
        _enable_profile_hook()
    res = run_bass_kernel_spmd(nc, in_maps, list(range(N_CORES)), trace=trace)
    out = np.concatenate([res.results[c]["out"] for c in range(N_CORES)],
                         axis=1).reshape(1, C, N).astype(np.float32)
    return out, res


def kernel(**inputs):
    out, _ = _run(inputs, trace=False)
    return out
